# revision 14
# baseline (speedup 1.0000x reference)
"""Trainium2 Bass kernel for MinimalLinearAttention.

  q = relu(x @ q_w.T + q_b); k = relu(x @ k_w.T + k_b); v = x @ v_w.T + v_b
  kv[b,h] = sum_s k[b,s,h,:] outer v[b,s,h,:]          (per batch, all tokens)
  out[b,s,h] = q[b,s,h,:] @ kv[b,h]
  y = out @ o_w.T + o_b

Sharding: token-parallel over 8 cores. Each core takes a 512-token slice of
every batch (2048 tokens), computes k/v projections + partial kv, AllReduces
kv across cores (per batch, overlapped with compute), then does the q
readout + output projection for its own tokens. Host concatenates slices.

bf16 v4: all matmul operands bf16 (host-cast), every weight resident in
SBUF, DMA order matched to the PE's consumption order, k/v biases applied
by DVE broadcast-add at PSUM eviction (no bias matmuls), per-pair kv
matmuls at N=128 into two PSUM banks, bf16 kv collective whose diagonal
blocks DMA straight into long-lived zeroed kvsb tiles (no on-device cast,
nothing in an engine stream ever waits on the collective), output computed
as Y^T so the o-bias fuses into the activation eviction (host transposes),
kv matmuls software-pipelined one token-tile behind the K/V projections,
and stage-2 ordered (kv DMAs -> q projections -> readouts -> y) per batch
so the last batch's AllReduce latency is hidden behind ~90us of compute.

On-device layouts (per core):
  xt   = x_slice.T            [D=1024, T=2048]   (T cols batch-major: b*512+s)
  wq/wk/wv/wo = W.T           [Din=1024, Dout=1024]
  K, V                        [T, D]     (from xt-stationary matmuls)
  Q^T                         [D, T]     (from w-stationary matmuls)
  kv per (batch, head-pair)   [128, 128] block-diagonal (2 heads of 64)
  O^T                         [D, T]
  Y^T                         [D, T]     (f32; host transposes)
"""

import os
import sys

os.environ.setdefault("MYCRO_LOCAL_CACHE", "1")

for _p in ("/opt/trn_rl_repo", "/root/.axon_site/_ro/trn_rl_repo"):
    if os.path.isdir(_p) and _p not in sys.path:
        sys.path.insert(0, _p)

import numpy as np

B, S, D, H, HD = 4, 4096, 1024, 16, 64
NCORES = 8
SC = S // NCORES          # 512 tokens per core per batch
T = B * SC                # 2048 rows per core
NPAIR = 8                 # head pairs (2 heads of 64 dims = 128 partitions)
NDIN = D // 128           # 8 Din tiles
NT = T // 128             # 16 T tiles per core
NTB = SC // 128           # 4 T tiles per batch

_CACHE = {}


def build_program_bf16():
    """bf16 v4 (see module docstring)."""
    if "nc_bf16" in _CACHE:
        return _CACHE["nc_bf16"]

    import concourse.bacc as bacc
    import concourse.tile as tile
    from concourse import bass, mybir

    f32 = mybir.dt.float32
    BF = mybir.dt.bfloat16
    RELU = mybir.ActivationFunctionType.Relu
    IDENT = mybir.ActivationFunctionType.Identity
    COPY = mybir.ActivationFunctionType.Copy
    ADD = mybir.AluOpType.add

    nc = bacc.Bacc("TRN2", target_bir_lowering=False, debug=False,
                   num_devices=NCORES)

    xt_d = nc.dram_tensor("xt", [D, T], BF, kind="ExternalInput").ap()
    wq_d = nc.dram_tensor("wq", [D, D], BF, kind="ExternalInput").ap()
    wk_d = nc.dram_tensor("wk", [D, D], BF, kind="ExternalInput").ap()
    wv_d = nc.dram_tensor("wv", [D, D], BF, kind="ExternalInput").ap()
    wo_d = nc.dram_tensor("wo", [D, D], BF, kind="ExternalInput").ap()
    bq_d = nc.dram_tensor("bq", [128, NDIN], f32, kind="ExternalInput").ap()
    bo_d = nc.dram_tensor("bo", [128, NDIN], f32, kind="ExternalInput").ap()
    bk_d = nc.dram_tensor("bk", [1, D], BF, kind="ExternalInput").ap()
    bv_d = nc.dram_tensor("bv", [1, D], BF, kind="ExternalInput").ap()
    y_d = nc.dram_tensor("y", [D, T], f32, kind="ExternalOutput").ap()

    HPB = 16 * 64  # bounce rows per batch: 16 heads x 64 d-rows

    with tile.TileContext(nc) as tc:
        with (
            tc.tile_pool(name="const", bufs=1) as constp,
            tc.tile_pool(name="wp", bufs=1) as wp,
            tc.tile_pool(name="xtp", bufs=1) as xtp,
            tc.tile_pool(name="kvb", bufs=1) as kvbp,
            tc.tile_pool(name="ktmp", bufs=3) as ktmpp,
            tc.tile_pool(name="qt", bufs=2) as qtp,
            tc.tile_pool(name="otb", bufs=2) as otbp,
            tc.tile_pool(name="kvex", bufs=8) as kvexp,
            tc.tile_pool(name="kvsb", bufs=1) as kvsbp,
            tc.tile_pool(name="yt", bufs=3) as ytp,
            tc.tile_pool(name="dram", bufs=1, space="DRAM") as dramp,
            tc.tile_pool(name="ps", bufs=5, space="PSUM") as psp,
            tc.tile_pool(name="pskv", bufs=2, space="PSUM") as pskvp,
        ):
            # ---- loads (program order = scheduling priority) ----
            ones = constp.tile([1, 128], BF, tag="ones")
            nc.vector.memset(ones[:], 1.0)
            bk_sb = constp.tile([1, D], BF, tag="bk")
            nc.sync.dma_start(bk_sb[:], bk_d[:])
            bv_sb = constp.tile([1, D], BF, tag="bv")
            nc.sync.dma_start(bv_sb[:], bv_d[:])

            # PE warm-up: cheap dummy matmuls keep the PE continuously busy
            # through the initial DMA window so the HAM clock-gate releases
            # (1.2 -> 2.4 GHz) before the real chains start
            warm = pskvp.tile([128, 512], f32, tag="kvps", name="warm")
            for i in range(60):
                nc.tensor.matmul(warm[:, 0:128], ones[:, 0:128],
                                 bk_sb[:, 0:128], start=True, stop=True)

            # broadcast k/v biases to all partitions (f32) via tiny matmuls
            bkb = constp.tile([128, D], f32, tag="bkb")
            bvb = constp.tile([128, D], f32, tag="bvb")
            for row, dst in ((bk_sb, bkb), (bv_sb, bvb)):
                for hf in range(2):
                    ps = psp.tile([128, 512], f32, tag="ps")
                    nc.tensor.matmul(ps[:], ones[:, 0:128],
                                     row[:, hf * 512:(hf + 1) * 512],
                                     start=True, stop=True)
                    nc.scalar.activation(dst[:, hf * 512:(hf + 1) * 512],
                                         ps[:], COPY)

            # zeroed block-diag kv holders (ping-pong per pair, stage 2)
            kvsb_pp = []
            for p in range(NPAIR):
                pair = []
                for g in range(2):
                    t = kvsbp.tile([128, 128], BF, tag=f"kvsb{p}_{g}",
                                   name=f"kvsb{p}_{g}")
                    nc.vector.memset(t[:], 0.0)
                    pair.append(t)
                kvsb_pp.append(pair)

            # first-needed set: xt cols 0:256, wk half 0, then the rest
            xts = []
            for dn in range(NDIN):
                t = xtp.tile([128, T], BF, tag=f"xt{dn}", name=f"xt_sb{dn}")
                nc.sync.dma_start(t[:, 0:256],
                                  xt_d[dn * 128:(dn + 1) * 128, 0:256])
                xts.append(t)
            wk_sb = []
            wv_sb = []
            for w_d, lst, tag in ((wk_d, wk_sb, "wk"), (wv_d, wv_sb, "wv")):
                for dn in range(NDIN):
                    t = wp.tile([128, D], BF, tag=f"{tag}{dn}",
                                name=f"{tag}_sb{dn}")
                    nc.sync.dma_start(t[:, 0:512],
                                      w_d[dn * 128:(dn + 1) * 128, 0:512])
                    lst.append(t)
            for dn in range(NDIN):
                nc.sync.dma_start(xts[dn][:, 256:512],
                                  xt_d[dn * 128:(dn + 1) * 128, 256:512])
            for w_d, lst in ((wk_d, wk_sb), (wv_d, wv_sb)):
                for dn in range(NDIN):
                    nc.sync.dma_start(lst[dn][:, 512:D],
                                      w_d[dn * 128:(dn + 1) * 128, 512:D])
            for c in range(1, 4):
                for dn in range(NDIN):
                    nc.sync.dma_start(
                        xts[dn][:, c * 512:(c + 1) * 512],
                        xt_d[dn * 128:(dn + 1) * 128, c * 512:(c + 1) * 512])

            def loadw(dram_ap, tag):
                w = []
                for dn in range(NDIN):
                    t = wp.tile([128, D], BF, tag=f"{tag}{dn}",
                                name=f"{tag}_sb{dn}")
                    nc.sync.dma_start(t[:], dram_ap[dn * 128:(dn + 1) * 128, :])
                    w.append(t)
                return w

            wq_sb = loadw(wq_d, "wq")
            bq_sb = constp.tile([128, NDIN], f32, tag="bq")
            nc.sync.dma_start(bq_sb[:], bq_d[:])
            wo_sb = loadw(wo_d, "wo")
            bo_sb = constp.tile([128, NDIN], f32, tag="bo")
            nc.sync.dma_start(bo_sb[:], bo_d[:])

            bnc_in = [dramp.tile([HPB, 64], BF, tag=f"bi{b}",
                                 name=f"bnc_in{b}") for b in range(B)]
            bnc_out = [dramp.tile([HPB, 64], BF, tag=f"bo{b}",
                                  addr_space="Shared", name=f"bnc_out{b}")
                       for b in range(B)]

            # ---- Stage 1: K,V projections + per-batch partial kv ----
            # Sweep order (all K-h0 chains, then V-h0, K-h1, V-h1, then the
            # kv matmuls): the first sweep depends only on xt-c0 + wk-h0 so
            # the PE streams without DMA stalls from the very start, and
            # evictions always finish well before the kv sweep reads them.
            for b in range(B):
                kvps = [pskvp.tile([128, 512], f32, tag="kvps",
                                   name=f"kvps{b}_{w}") for w in range(2)]
                kts = [kvbp.tile([128, D], BF, tag=f"kb{t}",
                                 name=f"kt{b}_{t}") for t in range(NTB)]
                vts = [kvbp.tile([128, D], BF, tag=f"vb{t}",
                                 name=f"vt{b}_{t}") for t in range(NTB)]

                for hf in range(2):
                    for t in range(NTB):  # K half-sweep
                        gt = b * NTB + t
                        ps = psp.tile([128, 512], f32, tag="ps")
                        for dn in range(NDIN):
                            nc.tensor.matmul(
                                ps[:],
                                xts[dn][:, gt * 128:(gt + 1) * 128],
                                wk_sb[dn][:, hf * 512:(hf + 1) * 512],
                                start=(dn == 0), stop=(dn == NDIN - 1))
                        ktmp = ktmpp.tile([128, 512], BF, tag="ktmp")
                        nc.vector.scalar_tensor_tensor(
                            ktmp[:], ps[:], 0.0,
                            bkb[:, hf * 512:(hf + 1) * 512], ADD, ADD)
                        nc.scalar.activation(
                            kts[t][:, hf * 512:(hf + 1) * 512], ktmp[:], RELU)
                    for t in range(NTB):  # V half-sweep
                        gt = b * NTB + t
                        ps = psp.tile([128, 512], f32, tag="ps")
                        for dn in range(NDIN):
                            nc.tensor.matmul(
                                ps[:],
                                xts[dn][:, gt * 128:(gt + 1) * 128],
                                wv_sb[dn][:, hf * 512:(hf + 1) * 512],
                                start=(dn == 0), stop=(dn == NDIN - 1))
                        nc.vector.scalar_tensor_tensor(
                            vts[t][:, hf * 512:(hf + 1) * 512], ps[:], 0.0,
                            bvb[:, hf * 512:(hf + 1) * 512], ADD, ADD)

                for t in range(NTB):  # kv sweep
                    for p in range(NPAIR):
                        c0 = (p % 4) * 128
                        nc.tensor.matmul(
                            kvps[p // 4][:, c0:c0 + 128],
                            kts[t][:, p * 128:(p + 1) * 128],
                            vts[t][:, p * 128:(p + 1) * 128],
                            start=(t == 0 and p % 4 == 0),
                            stop=(t == NTB - 1 and p % 4 == 3))

                # ship only the diagonal [64,64] blocks (head h = 2p+j)
                for p in range(NPAIR):
                    for j in range(2):
                        ex = kvexp.tile([64, 64], BF, tag="kvex",
                                        name=f"kvex{b}_{p}_{j}")
                        c0 = (p % 4) * 128 + j * 64
                        nc.vector.tensor_copy(
                            ex[:],
                            kvps[p // 4][j * 64:(j + 1) * 64, c0:c0 + 64])
                        h = 2 * p + j
                        nc.sync.dma_start(
                            bnc_in[b][h * 64:(h + 1) * 64, :], ex[:])
                nc.gpsimd.collective_compute(
                    "AllReduce", mybir.AluOpType.add,
                    replica_groups=[list(range(NCORES))],
                    ins=[bnc_in[b].opt()], outs=[bnc_out[b].opt()])

            # ---- Stage 2: per batch: kv DMAs, Q^T proj, readout, Y^T ----
            for b in range(B):
                # diagonal kv blocks land straight in the zeroed bf16
                # holders; only these DMAs (no engine ops) wait on the
                # collective
                for p in range(NPAIR):
                    for j in range(2):
                        h = 2 * p + j
                        nc.sync.dma_start(
                            kvsb_pp[p][b % 2][j * 64:(j + 1) * 64,
                                              j * 64:(j + 1) * 64],
                            bnc_out[b][h * 64:(h + 1) * 64, :])

                qts = []
                for p in range(NPAIR):
                    ps = psp.tile([128, 512], f32, tag="ps")
                    for dn in range(NDIN):
                        nc.tensor.matmul(
                            ps[:],
                            wq_sb[dn][:, p * 128:(p + 1) * 128],
                            xts[dn][:, b * 512:(b + 1) * 512],
                            start=(dn == 0), stop=(dn == NDIN - 1))
                    qt = qtp.tile([128, 512], BF, tag=f"qt{p}",
                                  name=f"qt{b}_{p}")
                    nc.scalar.activation(qt[:], ps[:], RELU,
                                         bias=bq_sb[:, p:p + 1])
                    qts.append(qt)

                otbs = []
                for p in range(NPAIR):
                    pso = psp.tile([128, 512], f32, tag="ps")
                    nc.tensor.matmul(pso[:], kvsb_pp[p][b % 2][:], qts[p][:],
                                     start=True, stop=True)
                    otb = otbp.tile([128, 512], BF, tag=f"otb{p}",
                                    name=f"otb{b}_{p}")
                    nc.vector.tensor_copy(otb[:], pso[:])
                    otbs.append(otb)

                for do in range(NDIN):
                    ps = psp.tile([128, 512], f32, tag="ps")
                    for dn in range(NDIN):
                        nc.tensor.matmul(
                            ps[:],
                            wo_sb[dn][:, do * 128:(do + 1) * 128],
                            otbs[dn][:],
                            start=(dn == 0), stop=(dn == NDIN - 1))
                    yt = ytp.tile([128, 512], f32, tag="yt")
                    nc.scalar.activation(yt[:], ps[:], IDENT,
                                         bias=bo_sb[:, do:do + 1])
                    nc.sync.dma_start(
                        y_d[do * 128:(do + 1) * 128,
                            b * 512:(b + 1) * 512], yt[:])

    nc.compile()
    _CACHE["nc_bf16"] = nc
    return nc


# test.py compatibility: both names resolve to the bf16 build.
def build_program():
    return build_program_bf16()


def prepare_in_maps(x, q_w, q_b, k_w, k_b, v_w, v_b, o_w, o_b, dtype="bf16"):
    import ml_dtypes
    mmdt = ml_dtypes.bfloat16
    shared = {
        "wq": np.ascontiguousarray(q_w.T).astype(mmdt),
        "wk": np.ascontiguousarray(k_w.T).astype(mmdt),
        "wv": np.ascontiguousarray(v_w.T).astype(mmdt),
        "wo": np.ascontiguousarray(o_w.T).astype(mmdt),
        "bq": np.ascontiguousarray(
            q_b.reshape(NDIN, 128).T).astype(np.float32),
        "bo": np.ascontiguousarray(
            o_b.reshape(NDIN, 128).T).astype(np.float32),
        "bk": k_b.reshape(1, D).astype(mmdt),
        "bv": v_b.reshape(1, D).astype(mmdt),
    }
    in_maps = []
    for c in range(NCORES):
        xs = x[:, c * SC:(c + 1) * SC, :].reshape(T, D)
        m = dict(shared)
        m["xt"] = np.ascontiguousarray(xs.T).astype(mmdt)
        in_maps.append(m)
    return in_maps


def gather_output(results):
    y = np.empty((B, S, D), dtype=np.float32)
    for c in range(NCORES):
        yc = results[c]["y"]
        if yc.shape == (D, T):  # Y^T layout
            yc = yc.T
        y[:, c * SC:(c + 1) * SC, :] = yc.reshape(B, SC, D)
    return y


DTYPE = "bf16"


def run(inputs, trace=False, dtype=None, **kw):
    from concourse import bass_utils
    nc = build_program_bf16()
    in_maps = prepare_in_maps(**inputs)
    res = bass_utils.run_bass_kernel_spmd(
        nc, in_maps, core_ids=list(range(NCORES)), trace=trace, **kw)
    return gather_output(res.results), res


def kernel(**inputs):
    y, _ = run(inputs)
    return y


# revision 17
# speedup vs baseline: 1.0175x; 1.0175x over previous
"""Trainium2 Bass kernel for MinimalLinearAttention.

  q = relu(x @ q_w.T + q_b); k = relu(x @ k_w.T + k_b); v = x @ v_w.T + v_b
  kv[b,h] = sum_s k[b,s,h,:] outer v[b,s,h,:]          (per batch, all tokens)
  out[b,s,h] = q[b,s,h,:] @ kv[b,h]
  y = out @ o_w.T + o_b

Sharding: token-parallel over 8 cores. Each core takes a 512-token slice of
every batch (2048 tokens), computes k/v projections + partial kv, AllReduces
kv across cores (per batch, overlapped with compute), then does the q
readout + output projection for its own tokens. Host concatenates slices.

bf16 v4: all matmul operands bf16 (host-cast), every weight resident in
SBUF, DMA order matched to the PE's consumption order, k/v biases applied
by DVE broadcast-add at PSUM eviction (no bias matmuls), per-pair kv
matmuls at N=128 into two PSUM banks, bf16 kv collective whose diagonal
blocks DMA straight into long-lived zeroed kvsb tiles (no on-device cast,
nothing in an engine stream ever waits on the collective), output computed
as Y^T so the o-bias fuses into the activation eviction (host transposes),
kv matmuls software-pipelined one token-tile behind the K/V projections,
and stage-2 ordered (kv DMAs -> q projections -> readouts -> y) per batch
so the last batch's AllReduce latency is hidden behind ~90us of compute.

On-device layouts (per core):
  xt   = x_slice.T            [D=1024, T=2048]   (T cols batch-major: b*512+s)
  wq/wk/wv/wo = W.T           [Din=1024, Dout=1024]
  K, V                        [T, D]     (from xt-stationary matmuls)
  Q^T                         [D, T]     (from w-stationary matmuls)
  kv per (batch, head-pair)   [128, 128] block-diagonal (2 heads of 64)
  O^T                         [D, T]
  Y^T                         [D, T]     (f32; host transposes)
"""

import os
import sys

os.environ.setdefault("MYCRO_LOCAL_CACHE", "1")

for _p in ("/opt/trn_rl_repo", "/root/.axon_site/_ro/trn_rl_repo"):
    if os.path.isdir(_p) and _p not in sys.path:
        sys.path.insert(0, _p)

import numpy as np

B, S, D, H, HD = 4, 4096, 1024, 16, 64
NCORES = 8
SC = S // NCORES          # 512 tokens per core per batch
T = B * SC                # 2048 rows per core
NPAIR = 8                 # head pairs (2 heads of 64 dims = 128 partitions)
NDIN = D // 128           # 8 Din tiles
NT = T // 128             # 16 T tiles per core
NTB = SC // 128           # 4 T tiles per batch

_CACHE = {}


def build_program_bf16():
    """bf16 v4 (see module docstring)."""
    if "nc_bf16" in _CACHE:
        return _CACHE["nc_bf16"]

    import concourse.bacc as bacc
    import concourse.tile as tile
    from concourse import bass, mybir

    f32 = mybir.dt.float32
    BF = mybir.dt.bfloat16
    RELU = mybir.ActivationFunctionType.Relu
    IDENT = mybir.ActivationFunctionType.Identity
    COPY = mybir.ActivationFunctionType.Copy
    ADD = mybir.AluOpType.add

    nc = bacc.Bacc("TRN2", target_bir_lowering=False, debug=False,
                   num_devices=NCORES)

    xt_d = nc.dram_tensor("xt", [D, T], BF, kind="ExternalInput").ap()
    wq_d = nc.dram_tensor("wq", [D, D], BF, kind="ExternalInput").ap()
    wk_d = nc.dram_tensor("wk", [D, D], BF, kind="ExternalInput").ap()
    wv_d = nc.dram_tensor("wv", [D, D], BF, kind="ExternalInput").ap()
    wo_d = nc.dram_tensor("wo", [D, D], BF, kind="ExternalInput").ap()
    bq_d = nc.dram_tensor("bq", [128, NDIN], f32, kind="ExternalInput").ap()
    bo_d = nc.dram_tensor("bo", [128, NDIN], f32, kind="ExternalInput").ap()
    bk_d = nc.dram_tensor("bk", [1, D], BF, kind="ExternalInput").ap()
    bv_d = nc.dram_tensor("bv", [1, D], BF, kind="ExternalInput").ap()
    y_d = nc.dram_tensor("y", [D, T], f32, kind="ExternalOutput").ap()

    HPB = 16 * 64  # bounce rows per batch: 16 heads x 64 d-rows

    with tile.TileContext(nc) as tc:
        with (
            tc.tile_pool(name="const", bufs=1) as constp,
            tc.tile_pool(name="wp", bufs=1) as wp,
            tc.tile_pool(name="xtp", bufs=1) as xtp,
            tc.tile_pool(name="kvb", bufs=1) as kvbp,
            tc.tile_pool(name="ktmp", bufs=3) as ktmpp,
            tc.tile_pool(name="qt", bufs=2) as qtp,
            tc.tile_pool(name="otb", bufs=2) as otbp,
            tc.tile_pool(name="kvex", bufs=8) as kvexp,
            tc.tile_pool(name="kvsb", bufs=1) as kvsbp,
            tc.tile_pool(name="yt", bufs=3) as ytp,
            tc.tile_pool(name="dram", bufs=1, space="DRAM") as dramp,
            tc.tile_pool(name="ps", bufs=5, space="PSUM") as psp,
            tc.tile_pool(name="pskv", bufs=2, space="PSUM") as pskvp,
        ):
            # ---- loads (program order = scheduling priority) ----
            ones = constp.tile([1, 128], BF, tag="ones")
            nc.vector.memset(ones[:], 1.0)
            bk_sb = constp.tile([1, D], BF, tag="bk")
            nc.sync.dma_start(bk_sb[:], bk_d[:])
            bv_sb = constp.tile([1, D], BF, tag="bv")
            nc.sync.dma_start(bv_sb[:], bv_d[:])

            # PE warm-up: cheap dummy matmuls keep the PE continuously busy
            # through the initial DMA window so the HAM clock-gate releases
            # (1.2 -> 2.4 GHz) before the real chains start
            warm = pskvp.tile([128, 512], f32, tag="kvps", name="warm")
            for i in range(76):
                nc.tensor.matmul(warm[:, 0:128], ones[:, 0:128],
                                 bk_sb[:, 0:128], start=True, stop=True)

            # broadcast k/v biases to all partitions (f32) via tiny matmuls
            bkb = constp.tile([128, D], f32, tag="bkb")
            bvb = constp.tile([128, D], f32, tag="bvb")
            for row, dst in ((bk_sb, bkb), (bv_sb, bvb)):
                for hf in range(2):
                    ps = psp.tile([128, 512], f32, tag="ps")
                    nc.tensor.matmul(ps[:], ones[:, 0:128],
                                     row[:, hf * 512:(hf + 1) * 512],
                                     start=True, stop=True)
                    nc.scalar.activation(dst[:, hf * 512:(hf + 1) * 512],
                                         ps[:], COPY)

            # zeroed block-diag kv holders (ping-pong per pair, stage 2)
            kvsb_pp = []
            for p in range(NPAIR):
                pair = []
                for g in range(2):
                    t = kvsbp.tile([128, 128], BF, tag=f"kvsb{p}_{g}",
                                   name=f"kvsb{p}_{g}")
                    nc.vector.memset(t[:], 0.0)
                    pair.append(t)
                kvsb_pp.append(pair)

            # first-needed set: xt cols 0:512, wk half 0, then the rest
            xts = []
            for dn in range(NDIN):
                t = xtp.tile([128, T], BF, tag=f"xt{dn}", name=f"xt_sb{dn}")
                nc.sync.dma_start(t[:, 0:512],
                                  xt_d[dn * 128:(dn + 1) * 128, 0:512])
                xts.append(t)
            wk_sb = []
            wv_sb = []
            for w_d, lst, tag in ((wk_d, wk_sb, "wk"), (wv_d, wv_sb, "wv")):
                for dn in range(NDIN):
                    t = wp.tile([128, D], BF, tag=f"{tag}{dn}",
                                name=f"{tag}_sb{dn}")
                    nc.sync.dma_start(t[:, 0:512],
                                      w_d[dn * 128:(dn + 1) * 128, 0:512])
                    lst.append(t)
            for w_d, lst in ((wk_d, wk_sb), (wv_d, wv_sb)):
                for dn in range(NDIN):
                    nc.sync.dma_start(lst[dn][:, 512:D],
                                      w_d[dn * 128:(dn + 1) * 128, 512:D])
            for c in range(1, 4):
                for dn in range(NDIN):
                    nc.sync.dma_start(
                        xts[dn][:, c * 512:(c + 1) * 512],
                        xt_d[dn * 128:(dn + 1) * 128, c * 512:(c + 1) * 512])

            def loadw(dram_ap, tag):
                w = []
                for dn in range(NDIN):
                    t = wp.tile([128, D], BF, tag=f"{tag}{dn}",
                                name=f"{tag}_sb{dn}")
                    nc.sync.dma_start(t[:], dram_ap[dn * 128:(dn + 1) * 128, :])
                    w.append(t)
                return w

            wq_sb = loadw(wq_d, "wq")
            bq_sb = constp.tile([128, NDIN], f32, tag="bq")
            nc.sync.dma_start(bq_sb[:], bq_d[:])
            wo_sb = loadw(wo_d, "wo")
            bo_sb = constp.tile([128, NDIN], f32, tag="bo")
            nc.sync.dma_start(bo_sb[:], bo_d[:])

            bnc_in = [dramp.tile([HPB, 64], BF, tag=f"bi{b}",
                                 name=f"bnc_in{b}") for b in range(B)]
            bnc_out = [dramp.tile([HPB, 64], BF, tag=f"bo{b}",
                                  addr_space="Shared", name=f"bnc_out{b}")
                       for b in range(B)]

            # ---- Stage 1: K,V projections + per-batch partial kv ----
            # Sweep order (all K-h0 chains, then V-h0, K-h1, V-h1, then the
            # kv matmuls): the first sweep depends only on xt-c0 + wk-h0 so
            # the PE streams without DMA stalls from the very start, and
            # evictions always finish well before the kv sweep reads them.
            for b in range(B):
                kvps = [pskvp.tile([128, 512], f32, tag="kvps",
                                   name=f"kvps{b}_{w}") for w in range(2)]
                kts = [kvbp.tile([128, D], BF, tag=f"kb{t}",
                                 name=f"kt{b}_{t}") for t in range(NTB)]
                vts = [kvbp.tile([128, D], BF, tag=f"vb{t}",
                                 name=f"vt{b}_{t}") for t in range(NTB)]

                for hf in range(2):
                    for t in range(NTB):  # K half-sweep
                        gt = b * NTB + t
                        ps = psp.tile([128, 512], f32, tag="ps")
                        for dn in range(NDIN):
                            nc.tensor.matmul(
                                ps[:],
                                xts[dn][:, gt * 128:(gt + 1) * 128],
                                wk_sb[dn][:, hf * 512:(hf + 1) * 512],
                                start=(dn == 0), stop=(dn == NDIN - 1))
                        ktmp = ktmpp.tile([128, 512], BF, tag="ktmp")
                        nc.vector.scalar_tensor_tensor(
                            ktmp[:], ps[:], 0.0,
                            bkb[:, hf * 512:(hf + 1) * 512], ADD, ADD)
                        nc.scalar.activation(
                            kts[t][:, hf * 512:(hf + 1) * 512], ktmp[:], RELU)
                    for t in range(NTB):  # V half-sweep
                        gt = b * NTB + t
                        ps = psp.tile([128, 512], f32, tag="ps")
                        for dn in range(NDIN):
                            nc.tensor.matmul(
                                ps[:],
                                xts[dn][:, gt * 128:(gt + 1) * 128],
                                wv_sb[dn][:, hf * 512:(hf + 1) * 512],
                                start=(dn == 0), stop=(dn == NDIN - 1))
                        nc.vector.scalar_tensor_tensor(
                            vts[t][:, hf * 512:(hf + 1) * 512], ps[:], 0.0,
                            bvb[:, hf * 512:(hf + 1) * 512], ADD, ADD)

                for t in range(NTB):  # kv sweep
                    for p in range(NPAIR):
                        c0 = (p % 4) * 128
                        nc.tensor.matmul(
                            kvps[p // 4][:, c0:c0 + 128],
                            kts[t][:, p * 128:(p + 1) * 128],
                            vts[t][:, p * 128:(p + 1) * 128],
                            start=(t == 0 and p % 4 == 0),
                            stop=(t == NTB - 1 and p % 4 == 3))

                # ship only the diagonal [64,64] blocks (head h = 2p+j)
                for p in range(NPAIR):
                    for j in range(2):
                        ex = kvexp.tile([64, 64], BF, tag="kvex",
                                        name=f"kvex{b}_{p}_{j}")
                        c0 = (p % 4) * 128 + j * 64
                        nc.vector.tensor_copy(
                            ex[:],
                            kvps[p // 4][j * 64:(j + 1) * 64, c0:c0 + 64])
                        h = 2 * p + j
                        nc.sync.dma_start(
                            bnc_in[b][h * 64:(h + 1) * 64, :], ex[:])
                nc.gpsimd.collective_compute(
                    "AllReduce", mybir.AluOpType.add,
                    replica_groups=[list(range(NCORES))],
                    ins=[bnc_in[b].opt()], outs=[bnc_out[b].opt()])

            # ---- Stage 2: per batch: kv DMAs, Q^T proj, readout, Y^T ----
            for b in range(B):
                # diagonal kv blocks land straight in the zeroed bf16
                # holders; only these DMAs (no engine ops) wait on the
                # collective
                for p in range(NPAIR):
                    for j in range(2):
                        h = 2 * p + j
                        nc.sync.dma_start(
                            kvsb_pp[p][b % 2][j * 64:(j + 1) * 64,
                                              j * 64:(j + 1) * 64],
                            bnc_out[b][h * 64:(h + 1) * 64, :])

                qts = []
                for p in range(NPAIR):
                    ps = psp.tile([128, 512], f32, tag="ps")
                    for dn in range(NDIN):
                        nc.tensor.matmul(
                            ps[:],
                            wq_sb[dn][:, p * 128:(p + 1) * 128],
                            xts[dn][:, b * 512:(b + 1) * 512],
                            start=(dn == 0), stop=(dn == NDIN - 1))
                    qt = qtp.tile([128, 512], BF, tag=f"qt{p}",
                                  name=f"qt{b}_{p}")
                    nc.scalar.activation(qt[:], ps[:], RELU,
                                         bias=bq_sb[:, p:p + 1])
                    qts.append(qt)

                otbs = []
                for p in range(NPAIR):
                    pso = psp.tile([128, 512], f32, tag="ps")
                    nc.tensor.matmul(pso[:], kvsb_pp[p][b % 2][:], qts[p][:],
                                     start=True, stop=True)
                    otb = otbp.tile([128, 512], BF, tag=f"otb{p}",
                                    name=f"otb{b}_{p}")
                    nc.vector.tensor_copy(otb[:], pso[:])
                    otbs.append(otb)

                for do in range(NDIN):
                    ps = psp.tile([128, 512], f32, tag="ps")
                    for dn in range(NDIN):
                        nc.tensor.matmul(
                            ps[:],
                            wo_sb[dn][:, do * 128:(do + 1) * 128],
                            otbs[dn][:],
                            start=(dn == 0), stop=(dn == NDIN - 1))
                    yt = ytp.tile([128, 512], f32, tag="yt")
                    nc.scalar.activation(yt[:], ps[:], IDENT,
                                         bias=bo_sb[:, do:do + 1])
                    nc.sync.dma_start(
                        y_d[do * 128:(do + 1) * 128,
                            b * 512:(b + 1) * 512], yt[:])

    nc.compile()
    _CACHE["nc_bf16"] = nc
    return nc


# test.py compatibility: both names resolve to the bf16 build.
def build_program():
    return build_program_bf16()


def prepare_in_maps(x, q_w, q_b, k_w, k_b, v_w, v_b, o_w, o_b, dtype="bf16"):
    import ml_dtypes
    mmdt = ml_dtypes.bfloat16
    shared = {
        "wq": np.ascontiguousarray(q_w.T).astype(mmdt),
        "wk": np.ascontiguousarray(k_w.T).astype(mmdt),
        "wv": np.ascontiguousarray(v_w.T).astype(mmdt),
        "wo": np.ascontiguousarray(o_w.T).astype(mmdt),
        "bq": np.ascontiguousarray(
            q_b.reshape(NDIN, 128).T).astype(np.float32),
        "bo": np.ascontiguousarray(
            o_b.reshape(NDIN, 128).T).astype(np.float32),
        "bk": k_b.reshape(1, D).astype(mmdt),
        "bv": v_b.reshape(1, D).astype(mmdt),
    }
    in_maps = []
    for c in range(NCORES):
        xs = x[:, c * SC:(c + 1) * SC, :].reshape(T, D)
        m = dict(shared)
        m["xt"] = np.ascontiguousarray(xs.T).astype(mmdt)
        in_maps.append(m)
    return in_maps


def gather_output(results):
    y = np.empty((B, S, D), dtype=np.float32)
    for c in range(NCORES):
        yc = results[c]["y"]
        if yc.shape == (D, T):  # Y^T layout
            yc = yc.T
        y[:, c * SC:(c + 1) * SC, :] = yc.reshape(B, SC, D)
    return y


DTYPE = "bf16"


def run(inputs, trace=False, dtype=None, **kw):
    from concourse import bass_utils
    nc = build_program_bf16()
    in_maps = prepare_in_maps(**inputs)
    res = bass_utils.run_bass_kernel_spmd(
        nc, in_maps, core_ids=list(range(NCORES)), trace=trace, **kw)
    return gather_output(res.results), res


def kernel(**inputs):
    y, _ = run(inputs)
    return y


# revision 18
# speedup vs baseline: 1.0188x; 1.0013x over previous
"""Trainium2 Bass kernel for MinimalLinearAttention.

  q = relu(x @ q_w.T + q_b); k = relu(x @ k_w.T + k_b); v = x @ v_w.T + v_b
  kv[b,h] = sum_s k[b,s,h,:] outer v[b,s,h,:]          (per batch, all tokens)
  out[b,s,h] = q[b,s,h,:] @ kv[b,h]
  y = out @ o_w.T + o_b

Sharding: token-parallel over 8 cores. Each core takes a 512-token slice of
every batch (2048 tokens), computes k/v projections + partial kv, AllReduces
kv across cores (per batch, overlapped with compute), then does the q
readout + output projection for its own tokens. Host concatenates slices.

bf16 final: all matmul operands bf16 (host-cast), every weight resident in
SBUF, DMA order matched to the PE's consumption order, k/v biases applied
by DVE broadcast-add at PSUM eviction (no bias matmuls), per-pair kv
matmuls at N=128 into two PSUM banks, bf16 kv collective whose diagonal
blocks DMA straight into long-lived zeroed kvsb tiles (no on-device cast,
nothing in an engine stream ever waits on the collective), output computed
as Y^T so the o-bias fuses into the activation eviction (host transposes),
stage 1 emitted as per-half sweeps (K-h0 x4, V-h0 x4, K-h1, V-h1, then the
kv sweep) so early PE work depends on minimal DMA, ~76 tiny warm-up
matmuls bridging the initial DMA window so the HAM clock-gate releases
before real chains start, and stage-2 ordered (kv DMAs -> q projections ->
readouts -> y) per batch so the last batch's AllReduce latency is hidden
behind ~90us of compute.

Measured: ~313-316 us HW exec (baseline 524 us), rel err 3.9e-3.
The remaining time is ~296 us of gap-free PE streaming at the k=13/16
SW power throttle (~1.95 GHz) + ~13.5 us fixed NEFF prologue/drain.

On-device layouts (per core):
  xt   = x_slice.T            [D=1024, T=2048]   (T cols batch-major: b*512+s)
  wq/wk/wv/wo = W.T           [Din=1024, Dout=1024]
  K, V                        [T, D]     (from xt-stationary matmuls)
  Q^T                         [D, T]     (from w-stationary matmuls)
  kv per (batch, head-pair)   [128, 128] block-diagonal (2 heads of 64)
  O^T                         [D, T]
  Y^T                         [D, T]     (f32; host transposes)
"""

import os
import sys

os.environ.setdefault("MYCRO_LOCAL_CACHE", "1")

for _p in ("/opt/trn_rl_repo", "/root/.axon_site/_ro/trn_rl_repo"):
    if os.path.isdir(_p) and _p not in sys.path:
        sys.path.insert(0, _p)

import numpy as np

B, S, D, H, HD = 4, 4096, 1024, 16, 64
NCORES = 8
SC = S // NCORES          # 512 tokens per core per batch
T = B * SC                # 2048 rows per core
NPAIR = 8                 # head pairs (2 heads of 64 dims = 128 partitions)
NDIN = D // 128           # 8 Din tiles
NT = T // 128             # 16 T tiles per core
NTB = SC // 128           # 4 T tiles per batch

_CACHE = {}


def build_program_bf16():
    """bf16 v4 (see module docstring)."""
    if "nc_bf16" in _CACHE:
        return _CACHE["nc_bf16"]

    import concourse.bacc as bacc
    import concourse.tile as tile
    from concourse import bass, mybir

    f32 = mybir.dt.float32
    BF = mybir.dt.bfloat16
    RELU = mybir.ActivationFunctionType.Relu
    IDENT = mybir.ActivationFunctionType.Identity
    COPY = mybir.ActivationFunctionType.Copy
    ADD = mybir.AluOpType.add

    nc = bacc.Bacc("TRN2", target_bir_lowering=False, debug=False,
                   num_devices=NCORES)

    xt_d = nc.dram_tensor("xt", [D, T], BF, kind="ExternalInput").ap()
    wq_d = nc.dram_tensor("wq", [D, D], BF, kind="ExternalInput").ap()
    wk_d = nc.dram_tensor("wk", [D, D], BF, kind="ExternalInput").ap()
    wv_d = nc.dram_tensor("wv", [D, D], BF, kind="ExternalInput").ap()
    wo_d = nc.dram_tensor("wo", [D, D], BF, kind="ExternalInput").ap()
    bq_d = nc.dram_tensor("bq", [128, NDIN], f32, kind="ExternalInput").ap()
    bo_d = nc.dram_tensor("bo", [128, NDIN], f32, kind="ExternalInput").ap()
    bk_d = nc.dram_tensor("bk", [1, D], BF, kind="ExternalInput").ap()
    bv_d = nc.dram_tensor("bv", [1, D], BF, kind="ExternalInput").ap()
    y_d = nc.dram_tensor("y", [D, T], f32, kind="ExternalOutput").ap()

    HPB = 16 * 64  # bounce rows per batch: 16 heads x 64 d-rows

    with tile.TileContext(nc) as tc:
        with (
            tc.tile_pool(name="const", bufs=1) as constp,
            tc.tile_pool(name="wp", bufs=1) as wp,
            tc.tile_pool(name="xtp", bufs=1) as xtp,
            tc.tile_pool(name="kvb", bufs=1) as kvbp,
            tc.tile_pool(name="ktmp", bufs=3) as ktmpp,
            tc.tile_pool(name="qt", bufs=2) as qtp,
            tc.tile_pool(name="otb", bufs=2) as otbp,
            tc.tile_pool(name="kvex", bufs=8) as kvexp,
            tc.tile_pool(name="kvsb", bufs=1) as kvsbp,
            tc.tile_pool(name="yt", bufs=3) as ytp,
            tc.tile_pool(name="dram", bufs=1, space="DRAM") as dramp,
            tc.tile_pool(name="ps", bufs=5, space="PSUM") as psp,
            tc.tile_pool(name="pskv", bufs=2, space="PSUM") as pskvp,
        ):
            # ---- loads (program order = scheduling priority) ----
            ones = constp.tile([1, 128], BF, tag="ones")
            nc.vector.memset(ones[:], 1.0)
            bk_sb = constp.tile([1, D], BF, tag="bk")
            nc.sync.dma_start(bk_sb[:], bk_d[:])
            bv_sb = constp.tile([1, D], BF, tag="bv")
            nc.sync.dma_start(bv_sb[:], bv_d[:])

            # PE warm-up: cheap dummy matmuls keep the PE continuously busy
            # through the initial DMA window so the HAM clock-gate releases
            # (1.2 -> 2.4 GHz) before the real chains start
            warm = pskvp.tile([128, 512], f32, tag="kvps", name="warm")
            for i in range(76):
                nc.tensor.matmul(warm[:, 0:128], ones[:, 0:128],
                                 bk_sb[:, 0:128], start=True, stop=True)

            # broadcast k/v biases to all partitions (f32) via tiny matmuls
            bkb = constp.tile([128, D], f32, tag="bkb")
            bvb = constp.tile([128, D], f32, tag="bvb")
            for row, dst in ((bk_sb, bkb), (bv_sb, bvb)):
                for hf in range(2):
                    ps = psp.tile([128, 512], f32, tag="ps")
                    nc.tensor.matmul(ps[:], ones[:, 0:128],
                                     row[:, hf * 512:(hf + 1) * 512],
                                     start=True, stop=True)
                    nc.scalar.activation(dst[:, hf * 512:(hf + 1) * 512],
                                         ps[:], COPY)

            # zeroed block-diag kv holders (ping-pong per pair, stage 2)
            kvsb_pp = []
            for p in range(NPAIR):
                pair = []
                for g in range(2):
                    t = kvsbp.tile([128, 128], BF, tag=f"kvsb{p}_{g}",
                                   name=f"kvsb{p}_{g}")
                    nc.vector.memset(t[:], 0.0)
                    pair.append(t)
                kvsb_pp.append(pair)

            # first-needed set: xt cols 0:512, wk half 0, then the rest
            xts = []
            for dn in range(NDIN):
                t = xtp.tile([128, T], BF, tag=f"xt{dn}", name=f"xt_sb{dn}")
                nc.sync.dma_start(t[:, 0:512],
                                  xt_d[dn * 128:(dn + 1) * 128, 0:512])
                xts.append(t)
            wk_sb = []
            wv_sb = []
            for w_d, lst, tag in ((wk_d, wk_sb, "wk"), (wv_d, wv_sb, "wv")):
                for dn in range(NDIN):
                    t = wp.tile([128, D], BF, tag=f"{tag}{dn}",
                                name=f"{tag}_sb{dn}")
                    nc.sync.dma_start(t[:, 0:512],
                                      w_d[dn * 128:(dn + 1) * 128, 0:512])
                    lst.append(t)
            for w_d, lst in ((wk_d, wk_sb), (wv_d, wv_sb)):
                for dn in range(NDIN):
                    nc.sync.dma_start(lst[dn][:, 512:D],
                                      w_d[dn * 128:(dn + 1) * 128, 512:D])
            for c in range(1, 4):
                for dn in range(NDIN):
                    nc.sync.dma_start(
                        xts[dn][:, c * 512:(c + 1) * 512],
                        xt_d[dn * 128:(dn + 1) * 128, c * 512:(c + 1) * 512])

            def loadw(dram_ap, tag):
                w = []
                for dn in range(NDIN):
                    t = wp.tile([128, D], BF, tag=f"{tag}{dn}",
                                name=f"{tag}_sb{dn}")
                    nc.sync.dma_start(t[:], dram_ap[dn * 128:(dn + 1) * 128, :])
                    w.append(t)
                return w

            wq_sb = loadw(wq_d, "wq")
            bq_sb = constp.tile([128, NDIN], f32, tag="bq")
            nc.sync.dma_start(bq_sb[:], bq_d[:])
            wo_sb = loadw(wo_d, "wo")
            bo_sb = constp.tile([128, NDIN], f32, tag="bo")
            nc.sync.dma_start(bo_sb[:], bo_d[:])

            bnc_in = [dramp.tile([HPB, 64], BF, tag=f"bi{b}",
                                 name=f"bnc_in{b}") for b in range(B)]
            bnc_out = [dramp.tile([HPB, 64], BF, tag=f"bo{b}",
                                  addr_space="Shared", name=f"bnc_out{b}")
                       for b in range(B)]

            # ---- Stage 1: K,V projections + per-batch partial kv ----
            # Sweep order (all K-h0 chains, then V-h0, K-h1, V-h1, then the
            # kv matmuls): the first sweep depends only on xt-c0 + wk-h0 so
            # the PE streams without DMA stalls from the very start, and
            # evictions always finish well before the kv sweep reads them.
            for b in range(B):
                kvps = [pskvp.tile([128, 512], f32, tag="kvps",
                                   name=f"kvps{b}_{w}") for w in range(2)]
                kts = [kvbp.tile([128, D], BF, tag=f"kb{t}",
                                 name=f"kt{b}_{t}") for t in range(NTB)]
                vts = [kvbp.tile([128, D], BF, tag=f"vb{t}",
                                 name=f"vt{b}_{t}") for t in range(NTB)]

                for hf in range(2):
                    for t in range(NTB):  # K half-sweep
                        gt = b * NTB + t
                        ps = psp.tile([128, 512], f32, tag="ps")
                        for dn in range(NDIN):
                            nc.tensor.matmul(
                                ps[:],
                                xts[dn][:, gt * 128:(gt + 1) * 128],
                                wk_sb[dn][:, hf * 512:(hf + 1) * 512],
                                start=(dn == 0), stop=(dn == NDIN - 1))
                        ktmp = ktmpp.tile([128, 512], BF, tag="ktmp")
                        nc.vector.scalar_tensor_tensor(
                            ktmp[:], ps[:], 0.0,
                            bkb[:, hf * 512:(hf + 1) * 512], ADD, ADD)
                        nc.scalar.activation(
                            kts[t][:, hf * 512:(hf + 1) * 512], ktmp[:], RELU)
                    for t in range(NTB):  # V half-sweep
                        gt = b * NTB + t
                        ps = psp.tile([128, 512], f32, tag="ps")
                        for dn in range(NDIN):
                            nc.tensor.matmul(
                                ps[:],
                                xts[dn][:, gt * 128:(gt + 1) * 128],
                                wv_sb[dn][:, hf * 512:(hf + 1) * 512],
                                start=(dn == 0), stop=(dn == NDIN - 1))
                        nc.vector.scalar_tensor_tensor(
                            vts[t][:, hf * 512:(hf + 1) * 512], ps[:], 0.0,
                            bvb[:, hf * 512:(hf + 1) * 512], ADD, ADD)

                for t in range(NTB):  # kv sweep
                    for p in range(NPAIR):
                        c0 = (p % 4) * 128
                        nc.tensor.matmul(
                            kvps[p // 4][:, c0:c0 + 128],
                            kts[t][:, p * 128:(p + 1) * 128],
                            vts[t][:, p * 128:(p + 1) * 128],
                            start=(t == 0 and p % 4 == 0),
                            stop=(t == NTB - 1 and p % 4 == 3))

                # ship only the diagonal [64,64] blocks (head h = 2p+j)
                for p in range(NPAIR):
                    for j in range(2):
                        ex = kvexp.tile([64, 64], BF, tag="kvex",
                                        name=f"kvex{b}_{p}_{j}")
                        c0 = (p % 4) * 128 + j * 64
                        nc.vector.tensor_copy(
                            ex[:],
                            kvps[p // 4][j * 64:(j + 1) * 64, c0:c0 + 64])
                        h = 2 * p + j
                        nc.sync.dma_start(
                            bnc_in[b][h * 64:(h + 1) * 64, :], ex[:])
                nc.gpsimd.collective_compute(
                    "AllReduce", mybir.AluOpType.add,
                    replica_groups=[list(range(NCORES))],
                    ins=[bnc_in[b].opt()], outs=[bnc_out[b].opt()])

            # ---- Stage 2: per batch: kv DMAs, Q^T proj, readout, Y^T ----
            for b in range(B):
                # diagonal kv blocks land straight in the zeroed bf16
                # holders; only these DMAs (no engine ops) wait on the
                # collective
                for p in range(NPAIR):
                    for j in range(2):
                        h = 2 * p + j
                        nc.sync.dma_start(
                            kvsb_pp[p][b % 2][j * 64:(j + 1) * 64,
                                              j * 64:(j + 1) * 64],
                            bnc_out[b][h * 64:(h + 1) * 64, :])

                qts = []
                for p in range(NPAIR):
                    ps = psp.tile([128, 512], f32, tag="ps")
                    for dn in range(NDIN):
                        nc.tensor.matmul(
                            ps[:],
                            wq_sb[dn][:, p * 128:(p + 1) * 128],
                            xts[dn][:, b * 512:(b + 1) * 512],
                            start=(dn == 0), stop=(dn == NDIN - 1))
                    qt = qtp.tile([128, 512], BF, tag=f"qt{p}",
                                  name=f"qt{b}_{p}")
                    nc.scalar.activation(qt[:], ps[:], RELU,
                                         bias=bq_sb[:, p:p + 1])
                    qts.append(qt)

                otbs = []
                for p in range(NPAIR):
                    pso = psp.tile([128, 512], f32, tag="ps")
                    nc.tensor.matmul(pso[:], kvsb_pp[p][b % 2][:], qts[p][:],
                                     start=True, stop=True)
                    otb = otbp.tile([128, 512], BF, tag=f"otb{p}",
                                    name=f"otb{b}_{p}")
                    nc.vector.tensor_copy(otb[:], pso[:])
                    otbs.append(otb)

                for do in range(NDIN):
                    ps = psp.tile([128, 512], f32, tag="ps")
                    for dn in range(NDIN):
                        nc.tensor.matmul(
                            ps[:],
                            wo_sb[dn][:, do * 128:(do + 1) * 128],
                            otbs[dn][:],
                            start=(dn == 0), stop=(dn == NDIN - 1))
                    yt = ytp.tile([128, 512], f32, tag="yt")
                    nc.scalar.activation(yt[:], ps[:], IDENT,
                                         bias=bo_sb[:, do:do + 1])
                    nc.sync.dma_start(
                        y_d[do * 128:(do + 1) * 128,
                            b * 512:(b + 1) * 512], yt[:])

    nc.compile()
    _CACHE["nc_bf16"] = nc
    return nc


# test.py compatibility: both names resolve to the bf16 build.
def build_program():
    return build_program_bf16()


def prepare_in_maps(x, q_w, q_b, k_w, k_b, v_w, v_b, o_w, o_b, dtype="bf16"):
    import ml_dtypes
    mmdt = ml_dtypes.bfloat16
    shared = {
        "wq": np.ascontiguousarray(q_w.T).astype(mmdt),
        "wk": np.ascontiguousarray(k_w.T).astype(mmdt),
        "wv": np.ascontiguousarray(v_w.T).astype(mmdt),
        "wo": np.ascontiguousarray(o_w.T).astype(mmdt),
        "bq": np.ascontiguousarray(
            q_b.reshape(NDIN, 128).T).astype(np.float32),
        "bo": np.ascontiguousarray(
            o_b.reshape(NDIN, 128).T).astype(np.float32),
        "bk": k_b.reshape(1, D).astype(mmdt),
        "bv": v_b.reshape(1, D).astype(mmdt),
    }
    in_maps = []
    for c in range(NCORES):
        xs = x[:, c * SC:(c + 1) * SC, :].reshape(T, D)
        m = dict(shared)
        m["xt"] = np.ascontiguousarray(xs.T).astype(mmdt)
        in_maps.append(m)
    return in_maps


def gather_output(results):
    y = np.empty((B, S, D), dtype=np.float32)
    for c in range(NCORES):
        yc = results[c]["y"]
        if yc.shape == (D, T):  # Y^T layout
            yc = yc.T
        y[:, c * SC:(c + 1) * SC, :] = yc.reshape(B, SC, D)
    return y


DTYPE = "bf16"


def run(inputs, trace=False, dtype=None, **kw):
    from concourse import bass_utils
    nc = build_program_bf16()
    in_maps = prepare_in_maps(**inputs)
    res = bass_utils.run_bass_kernel_spmd(
        nc, in_maps, core_ids=list(range(NCORES)), trace=trace, **kw)
    return gather_output(res.results), res


def kernel(**inputs):
    y, _ = run(inputs)
    return y
